# revision 1
# baseline (speedup 1.0000x reference)
"""GAT multi-head attention layer (nn_GATMutiHeadAttLayer) on 8 Trainium2 cores.

Head-sharded: core h computes head h entirely (no collectives).

Math (per head):
  h = X @ W                       [S, FOUT]
  f1 = h @ a1, f2 = h @ a2        [S]
  e[i,j] = lrelu(f1[i] + f2[j], 0.2), masked by adj[i,j]
  attn = softmax(e, axis=i)  (column-normalized; softmax over source axis)
  out = attn @ h, concat heads, ELU.

Device formulation (transposed, j on partitions):
  P^T[j,i] = M^T[j,i] * max(exp(f1[i]+f2[j]), exp(0.2(f1[i]+f2[j])))
           = (Q = E1s[i]*M^T) * (u' = max(R1[i]*e2[j], e2s[j]))
    R1 = exp(0.8 f1), E1s = exp(0.2 f1), e2 = exp(f2), e2s = exp(0.2 f2)
  s[j] = sum_i P^T[j,i]    (ACT accumulate)
  out^T[o,i] = sum_j (h[j,o]/s[j]) * P^T[j,i]   (PE, PSUM-accumulated over bands)
  final: ELU(out^T) -> DRAM; host transposes/concats heads.

Host prep: X^T (fp32) and adj^T cast to fp16 (mask is 0/1 exactly), W/a1/a2
sliced per head. All "transposes" are host-side layout prep; all model compute
(matmuls, exp, masking, softmax, ELU) runs on device.
"""

import contextlib
import ctypes
import sys
import types
from contextlib import ExitStack

import numpy as np

import concourse.bass as bass
import concourse.tile as tile
from concourse import bacc, mybir
from concourse import bass_utils

AF = mybir.ActivationFunctionType
ALU = mybir.AluOpType
DT = mybir.dt

S = 4096
FIN = 128
FOUT = 64
H = 8
ALPHA = 0.2

LAST_RESULTS = None  # BassKernelResults of the most recent run (for test harness)

# ---------------------------------------------------------------------------
# NTFF profile hook shim: antenv.axon_hooks is absent in this container; the
# trace=True path of run_bass_kernel_spmd imports it. Recreate it via ctypes
# against libaxon_pjrt.so (same as trn_agent_boot does).
_SO_PATH = "/opt/axon/libaxon_pjrt.so"


def _make_ntff_hook():
    try:
        lib = ctypes.CDLL(_SO_PATH)
    except OSError:
        return None
    if not hasattr(lib, "axon_start_nrt_profile"):
        return None
    lib.axon_start_nrt_profile.argtypes = [ctypes.POINTER(ctypes.c_int64), ctypes.c_size_t]
    lib.axon_start_nrt_profile.restype = ctypes.c_int64
    lib.axon_stop_nrt_profile.argtypes = [ctypes.c_char_p]
    lib.axon_stop_nrt_profile.restype = ctypes.c_int64

    @contextlib.contextmanager
    def _hook(output_dir, device_ids):
        import jax

        jax.devices()
        if device_ids:
            ids = (ctypes.c_int64 * len(device_ids))(*device_ids)
            rc = lib.axon_start_nrt_profile(ids, len(device_ids))
        else:
            rc = lib.axon_start_nrt_profile(None, 0)
        if rc != 0:
            raise RuntimeError(f"axon_start_nrt_profile rc={rc}")
        try:
            yield
        finally:
            n = lib.axon_stop_nrt_profile(str(output_dir).encode())
            if n <= 0:
                print(f"ntff profile: rc={n} (no files?) dir={output_dir}", file=sys.stderr)

    return _hook


def _install_ntff_shim():
    if "antenv.axon_hooks" in sys.modules:
        return
    mod = types.ModuleType("antenv.axon_hooks")
    _hook = _make_ntff_hook()
    mod.get_axon_ntff_profile_hook = lambda: _hook
    mod.set_axon_ntff_profile_hook = lambda h: None
    sys.modules["antenv.axon_hooks"] = mod
    try:
        import antenv

        antenv.axon_hooks = mod
    except ImportError:
        pass


_install_ntff_shim()


def _enable_ldw_opt():
    """Walrus ships an LDWEIGHTS-dedup pass that concourse hardcodes off.
    Rewrites the walrus argv to enable it (correctness is validated by the
    caller against the reference)."""
    if getattr(bass_utils, "_ldwopt_patched", False):
        return
    orig = bass_utils.run_command

    def patched(cmd, *a, **kw):
        cmd = ["--enable-ldw-opt=true" if c == "--enable-ldw-opt=false" else c for c in cmd]
        return orig(cmd, *a, **kw)

    bass_utils.run_command = patched
    bass_utils._ldwopt_patched = True


import os as _os

if _os.environ.get("KERNEL_LDWOPT") == "1":
    _enable_ldw_opt()

# ---------------------------------------------------------------------------


def build_nc(s=S, microbench=False, variants=None):
    """Build + compile the per-core Bass program (same program on all cores)."""
    nb = s // 128     # number of j-bands
    nch = s // 512    # number of 512-wide i-chunks

    nc = bacc.Bacc("TRN2", target_bir_lowering=False, debug=False, enable_asserts=False)

    xt = nc.dram_tensor("xt", [FIN, s], DT.float32, kind="ExternalInput").ap()
    w = nc.dram_tensor("w", [FIN, FOUT], DT.float32, kind="ExternalInput").ap()
    a1 = nc.dram_tensor("a1", [FOUT, 1], DT.float32, kind="ExternalInput").ap()
    a2 = nc.dram_tensor("a2", [FOUT, 1], DT.float32, kind="ExternalInput").ap()
    adjt = nc.dram_tensor("adjt", [s, s], DT.float16, kind="ExternalInput").ap()
    out = nc.dram_tensor("out", [FOUT, s], DT.float32, kind="ExternalOutput").ap()

    if variants is None:
        variants = "A" * nb
    assert len(variants) == nb
    with tile.TileContext(nc) as tc, ExitStack() as ctx:
        _body(ctx, tc, nc, xt, w, a1, a2, adjt, out, s, nb, nch, microbench, variants)

    import os as _os

    if _os.environ.get("KERNEL_LDW1") == "1":
        # Mark matmuls whose stationary operand AP repeats the immediately
        # preceding matmul's as non-self-loading (PE keeps the loaded array).
        n_marked = 0
        for blk in nc.m.functions[0].blocks:
            prev_w = None
            for inst in blk.instructions:
                if type(inst).__name__ != "InstMatmult":
                    continue
                wkey = repr(inst.ins[1])
                if prev_w == wkey:
                    inst.ldweights = False
                    n_marked += 1
                prev_w = wkey
        print(f"KERNEL_LDW1: marked {n_marked} matmuls non-self-loading")

    nc.compile()
    return nc


def _body(ctx, tc, nc, xt, w, a1, a2, adjt, out, s, nb, nch, microbench, variants):
    variant_of = lambda b: variants[b]
    f32, f16 = DT.float32, DT.float16

    if microbench:
        # A few throwaway ops on dummy tiles so the HW trace reveals per-op
        # durations of the candidate DVE/ACT primitives at FD=4096.
        with tc.tile_pool(name="mb", bufs=1) as mbp:
            da = mbp.tile([128, 4096], f16, tag="da")
            db = mbp.tile([128, 4096], f16, tag="db")
            dc = mbp.tile([128, 4096], f16, tag="dc")
            dsc = mbp.tile([128, 1], f32, tag="dsc")
            dac = mbp.tile([128, 1], f32, tag="dac")
            nc.vector.memset(da[:], 1.0)
            nc.vector.memset(db[:], 0.5)
            nc.vector.memset(dsc[:], 2.0)
            # 1: plain TT mult (expect 2x)
            nc.vector.tensor_tensor(out=dc[:], in0=da[:], in1=db[:], op=ALU.mult)
            # 2: TS two-scalar (expect 4x)
            nc.vector.tensor_scalar(out=dc[:], in0=da[:], scalar1=dsc[:], scalar2=dsc[:], op0=ALU.mult, op1=ALU.max)
            # 3: TS with accum_out (needs a real 2nd op when reducing)
            nc.vector.tensor_scalar(out=dc[:], in0=da[:], scalar1=dsc[:], scalar2=1.0, op0=ALU.mult, op1=ALU.mult, accum_out=dac[:])
            # 4: STT mult+max (cost model says 1x; HW may be faster)
            nc.vector.scalar_tensor_tensor(out=dc[:], in0=da[:], scalar=dsc[:], in1=db[:], op0=ALU.mult, op1=ALU.max)
            # 5: STT with accum
            nc.vector.scalar_tensor_tensor(out=dc[:], in0=da[:], scalar=1.0, in1=db[:], op0=ALU.mult, op1=ALU.mult, accum_out=dac[:])
            # 6: TTR fused (cost model 1x)
            nc.vector.tensor_tensor_reduce(out=dc[:], in0=da[:], in1=db[:], scale=1.0, scalar=0.0, op0=ALU.mult, op1=ALU.add, accum_out=dac[:])
            # 7: ACT copy with accum
            nc.scalar.activation(dc[:], da[:], AF.Copy, accum_out=dac[:])

    # ---------------- persistent intermediates (live through main loop) ----
    cpool = ctx.enter_context(tc.tile_pool(name="const", bufs=1))
    h_sb = cpool.tile([128, nb * FOUT], f16, tag="h")     # h natural fp16, band b at cols [b*64, +64)
    eu_sb = cpool.tile([128, nb], f32, tag="eu")          # exp(f2)
    ev_sb = cpool.tile([128, nb], f32, tag="ev")          # exp(0.2 f2)
    need_a = "A" in variants
    need_b = "B" in variants
    r1b_sb = None
    e1b_sb = None
    if need_a:
        r1b_sb = cpool.tile([128, s], f16, tag="r1b")     # broadcast exp(0.8 f1)
    e1sb_sb = cpool.tile([128, s], f16, tag="e1sb")       # broadcast exp(0.2 f1)
    if need_b:
        e1b_sb = cpool.tile([128, s], f16, tag="e1b")     # broadcast exp(f1)

    # ---------------- preamble (scoped pool, freed before main loop) -------
    with tc.tile_pool(name="pre_sb", bufs=1) as tpool:
        xt_sb = tpool.tile([FIN, s], f32, tag="xt")
        nc.sync.dma_start(xt_sb[:], xt[:])
        w_sb = tpool.tile([FIN, FOUT], f32, tag="w")
        nc.sync.dma_start(w_sb[:], w[:])
        a1_sb = tpool.tile([FOUT, 1], f32, tag="a1")
        nc.sync.dma_start(a1_sb[:], a1[:])
        a2_sb = tpool.tile([FOUT, 1], f32, tag="a2")
        nc.sync.dma_start(a2_sb[:], a2[:])
        ones_sb = tpool.tile([1, 128], f32, tag="ones")
        nc.vector.memset(ones_sb[:], 1.0)
        ht_sb = tpool.tile([FOUT, s], f32, tag="ht")      # h^T
        f1_sb = tpool.tile([1, s], f32, tag="f1")         # f1 row
        f2_sb = tpool.tile([128, nb], f32, tag="f2")      # f2, band b in col b

        # phase 1: h^T and h
        with tc.tile_pool(name="pre1", bufs=2, space="PSUM") as pp1:
            for c in range(s // 512):
                pht = pp1.tile([FOUT, 512], f32, tag="ht")
                nc.tensor.matmul(pht[:], lhsT=w_sb[:], rhs=xt_sb[:, bass.ts(c, 512)], start=True, stop=True)
                nc.scalar.activation(ht_sb[:, bass.ts(c, 512)], pht[:], AF.Copy)
            for b in range(nb):
                ph = pp1.tile([128, FOUT], f32, tag="h")
                nc.tensor.matmul(ph[:], lhsT=xt_sb[:, bass.ts(b, 128)], rhs=w_sb[:], start=True, stop=True)
                nc.scalar.activation(h_sb[:, bass.ts(b, FOUT)], ph[:], AF.Copy)

        # phase 2: f1 row, f2 cols
        with tc.tile_pool(name="pre2", bufs=2, space="PSUM") as pp2:
            for c in range(s // 512):
                pf1 = pp2.tile([1, 512], f32, tag="f1")
                nc.tensor.matmul(pf1[:], lhsT=a1_sb[:], rhs=ht_sb[:, bass.ts(c, 512)], start=True, stop=True)
                nc.scalar.activation(f1_sb[:, bass.ts(c, 512)], pf1[:], AF.Copy)
            for b in range(nb):
                pf2 = pp2.tile([128, 1], f32, tag="f2")
                nc.tensor.matmul(pf2[:], lhsT=ht_sb[:, bass.ts(b, 128)], rhs=a2_sb[:], start=True, stop=True)
                nc.scalar.activation(f2_sb[:, b : b + 1], pf2[:], AF.Copy)

        # exp of f2 cols (small ACT ops)
        nc.scalar.activation(eu_sb[:], f2_sb[:], AF.Exp)
        nc.scalar.activation(ev_sb[:], f2_sb[:], AF.Exp, scale=0.2)

        # phase 3: broadcast f1 row to 128 partitions; exp folded into the
        # PSUM->SBUF copy (Exp with scale applied on the activation).
        with tc.tile_pool(name="pre3", bufs=2, space="PSUM") as pp3:
            for c in range(s // 512):
                pb1 = pp3.tile([128, 512], f32, tag="bc")
                nc.tensor.matmul(pb1[:], lhsT=ones_sb[:], rhs=f1_sb[:, bass.ts(c, 512)], start=True, stop=True)
                if need_a:
                    nc.scalar.activation(r1b_sb[:, bass.ts(c, 512)], pb1[:], AF.Exp, scale=0.8)
                nc.scalar.activation(e1sb_sb[:, bass.ts(c, 512)], pb1[:], AF.Exp, scale=0.2)
                if need_b:
                    nc.scalar.activation(e1b_sb[:, bass.ts(c, 512)], pb1[:], AF.Exp)

    # ---------------- main loop over j-bands ----------------
    mpool = ctx.enter_context(tc.tile_pool(name="mask", bufs=4))
    upool = ctx.enter_context(tc.tile_pool(name="umax", bufs=2))
    qpool = ctx.enter_context(tc.tile_pool(name="qmsk", bufs=2))
    ppool = ctx.enter_context(tc.tile_pool(name="pmat", bufs=3))
    scrpool = ctx.enter_context(tc.tile_pool(name="scr", bufs=2))
    spool = ctx.enter_context(tc.tile_pool(name="svec", bufs=6))
    hppool = ctx.enter_context(tc.tile_pool(name="hp", bufs=3))

    mainpsum = ctx.enter_context(tc.tile_pool(name="out_psum", bufs=1, space="PSUM"))
    psum_out = mainpsum.tile([FOUT, s], f32, tag="out")

    for b in range(nb):
        m_t = mpool.tile([128, s], f16, tag="m")
        nc.sync.dma_start(m_t[:], adjt[bass.ts(b, 128), :])

        s_t = spool.tile([128, 1], f32, tag="s")
        p_t = ppool.tile([128, s], f16, tag="p")
        if variant_of(b) == "A":
            # u' = max(R1[i]*e2[j], e2s[j])   (TS, 4x)
            u_t = upool.tile([128, s], f16, tag="u")
            nc.vector.tensor_scalar(
                out=u_t[:], in0=r1b_sb[:], scalar1=eu_sb[:, b : b + 1],
                scalar2=ev_sb[:, b : b + 1], op0=ALU.mult, op1=ALU.max,
            )
            # Q = E1s[i] * M   (TT, 2x)
            q_t = qpool.tile([128, s], f16, tag="q")
            nc.vector.tensor_tensor(out=q_t[:], in0=e1sb_sb[:], in1=m_t[:], op=ALU.mult)
            # P = u' * Q       (TT, 2x)
            nc.vector.tensor_tensor(out=p_t[:], in0=u_t[:], in1=q_t[:], op=ALU.mult)

            # s[j] = sum_i P (ACT pass with accumulator; scratch output)
            scr_t = scrpool.tile([128, s], f16, tag="scr")
            nc.scalar.activation(scr_t[:], p_t[:], AF.Copy, accum_out=s_t[:])
        else:
            # Variant B: ACT makes the v-arm; two STT ops do the rest.
            # v = E1s[i]*e2s[j]  (ACT copy with per-partition scale)
            v_t = upool.tile([128, s], f16, tag="u")
            nc.scalar.activation(v_t[:], e1sb_sb[:], AF.Copy, scale=ev_sb[:, b : b + 1])
            # w = max(E1[i]*e2[j], v) = (R1b*E1sb...)  -- use E1b = R1b*E1sb?
            # w = max(R1[i]*e2[j], ...)*E1s[i] is variant A's split; here use
            # full E1 row: w = max(E1b*eu[j], v)  (STT)
            w_t = qpool.tile([128, s], f16, tag="q")
            nc.vector.scalar_tensor_tensor(
                out=w_t[:], in0=e1b_sb[:], scalar=eu_sb[:, b : b + 1], in1=v_t[:],
                op0=ALU.mult, op1=ALU.max,
            )
            # P = w*M with fused row-sum accumulation (STT + accum)
            nc.vector.scalar_tensor_tensor(
                out=p_t[:], in0=w_t[:], scalar=1.0, in1=m_t[:],
                op0=ALU.mult, op1=ALU.mult, accum_out=s_t[:],
            )

        # hp = h[band] / s  -> fp16
        rs_t = spool.tile([128, 1], f32, tag="rs")
        nc.vector.reciprocal(rs_t[:], s_t[:])
        hp_t = hppool.tile([128, FOUT], f16, tag="hp")
        nc.vector.tensor_scalar_mul(out=hp_t[:], in0=h_sb[:, bass.ts(b, FOUT)], scalar1=rs_t[:])

        # out^T[o, i] += sum_j hp[j, o] * P[j, i]
        for c in range(nch):
            nc.tensor.matmul(
                psum_out[:, bass.ts(c, 512)], lhsT=hp_t[:], rhs=p_t[:, bass.ts(c, 512)],
                start=(b == 0), stop=(b == nb - 1),
            )

    # ---------------- ELU + writeout (chunked to bound SBUF) ----------------
    fpool = ctx.enter_context(tc.tile_pool(name="fin", bufs=2))
    ew = min(2048, s)
    for c in range(s // ew):
        sl = bass.ts(c, ew)
        r_t = fpool.tile([FOUT, ew], f32, tag="relu")
        nc.scalar.activation(r_t[:], psum_out[:, sl], AF.Relu)
        mn_t = fpool.tile([FOUT, ew], f32, tag="min")
        nc.vector.tensor_scalar_min(out=mn_t[:], in0=psum_out[:, sl], scalar1=0.0)
        e_t = fpool.tile([FOUT, ew], f32, tag="exp")
        nc.scalar.activation(e_t[:], mn_t[:], AF.Exp)
        f_t = fpool.tile([FOUT, ew], f32, tag="fin")
        # f = (e - 1) + r
        nc.vector.scalar_tensor_tensor(out=f_t[:], in0=e_t[:], scalar=-1.0, in1=r_t[:], op0=ALU.add, op1=ALU.add)
        nc.sync.dma_start(out[:, sl], f_t[:])


_NC_CACHE = {}


def _get_nc(s=S, microbench=False, variants=None):
    key = (s, microbench, variants)
    if key not in _NC_CACHE:
        _NC_CACHE[key] = build_nc(s, microbench, variants)
    return _NC_CACHE[key]


def kernel(input_seq, adj, W, a_1, a_2):
    """Full-input entry point: shards by head across 8 cores, returns [S, H*FOUT]."""
    global LAST_RESULTS
    X = np.asarray(input_seq)[0]          # [S, FIN] f32
    adjm = np.asarray(adj)[0]             # [S, S] int32
    Wn = np.asarray(W)                    # [H, FIN, FOUT]
    a1n = np.asarray(a_1)                 # [H, FOUT, 1]
    a2n = np.asarray(a_2)                 # [H, FOUT, 1]

    s = X.shape[0]
    xt = np.ascontiguousarray(X.T, dtype=np.float32)
    adjt = np.ascontiguousarray((adjm.T != 0)).astype(np.float16)

    import os

    nc = _get_nc(s, microbench=os.environ.get("KERNEL_MICROBENCH") == "1",
                 variants=os.environ.get("KERNEL_VARIANTS"))
    in_maps = [
        {
            "xt": xt,
            "w": np.ascontiguousarray(Wn[h], dtype=np.float32),
            "a1": np.ascontiguousarray(a1n[h], dtype=np.float32),
            "a2": np.ascontiguousarray(a2n[h], dtype=np.float32),
            "adjt": adjt,
        }
        for h in range(H)
    ]
    res = bass_utils.run_bass_kernel_spmd(nc, in_maps, core_ids=list(range(H)))
    LAST_RESULTS = res

    outf = np.empty((s, H * FOUT), dtype=np.float32)
    for h in range(H):
        outf[:, h * FOUT : (h + 1) * FOUT] = res.results[h]["out"].T
    return outf



# revision 4
# speedup vs baseline: 1.0407x; 1.0407x over previous
"""GAT multi-head attention layer (nn_GATMutiHeadAttLayer) on 8 Trainium2 cores.

Head-sharded: core h computes head h entirely (no collectives).

Math (per head):
  h = X @ W                       [S, FOUT]
  f1 = h @ a1, f2 = h @ a2        [S]
  e[i,j] = lrelu(f1[i] + f2[j], 0.2), masked by adj[i,j]
  attn = softmax(e, axis=i)  (denominator s[j] = sum_i)
  out = attn @ h, concat heads, ELU.

Device formulation (transposed, j on partitions; all fp16 tiles):
  exp(lrelu(z)) = max(exp(z), exp(0.2 z)),  z = f1[i] + f2[j]
  factor by e2s[j] = exp(0.2 f2[j]):
    t[j,i]  = max(E1b[i] * et[j], E1sb[i])          (STT: mult, max)
      E1b = exp(f1[i]) bcast, E1sb = exp(0.2 f1[i]) bcast, et = exp(0.8 f2[j])
    P[j,i]  = (t * e2s[j]) * M^T[j,i], s[j] = sum_i P  (STT: mult, mult, accum)
  hp = h[band] / s[j]  (ACT copy with per-partition scale 1/s)
  out^T[o,i] = sum_j hp[j,o] * P[j,i]   (PE, PSUM-accumulated over bands)
  final: ELU(out^T) -> DRAM; host transposes/concats heads.

Preamble (all matmuls on PE, fp16):
  wa = W @ [a1|a2]  -> wa1, wa2        (one [128,2] matmul via W^T)
  f1 row = wa1^T @ X^T  (8 x N=512), bcast to 128 partitions via ones-matmul,
  exp'd straight out of PSUM (Exp / Exp scale=0.2) -> E1b, E1sb.
  [h_band | f2_band] = xt_band^T @ [W | wa2]  (one N=65 matmul per band)
  et = exp(0.8 f2), e2s = exp(0.2 f2).

Host prep: X^T, W, W^T, [a1|a2] cast fp16; adj^T cast to fp16 (0/1 exact).
All model compute (matmuls, exp, masking, softmax, ELU) runs on device.
"""

import contextlib
import ctypes
import os
import sys
import types
from contextlib import ExitStack

import numpy as np

import concourse.bass as bass
import concourse.tile as tile
from concourse import bacc, mybir
from concourse import bass_utils

AF = mybir.ActivationFunctionType
ALU = mybir.AluOpType
DT = mybir.dt

S = 4096
FIN = 128
FOUT = 64
H = 8
ALPHA = 0.2

LAST_RESULTS = None  # BassKernelResults of the most recent run (for test harness)

# ---------------------------------------------------------------------------
# NTFF profile hook shim: antenv.axon_hooks is absent in this container; the
# trace=True path of run_bass_kernel_spmd imports it. Recreate it via ctypes
# against libaxon_pjrt.so (same as trn_agent_boot does).
_SO_PATH = "/opt/axon/libaxon_pjrt.so"


def _make_ntff_hook():
    try:
        lib = ctypes.CDLL(_SO_PATH)
    except OSError:
        return None
    if not hasattr(lib, "axon_start_nrt_profile"):
        return None
    lib.axon_start_nrt_profile.argtypes = [ctypes.POINTER(ctypes.c_int64), ctypes.c_size_t]
    lib.axon_start_nrt_profile.restype = ctypes.c_int64
    lib.axon_stop_nrt_profile.argtypes = [ctypes.c_char_p]
    lib.axon_stop_nrt_profile.restype = ctypes.c_int64

    @contextlib.contextmanager
    def _hook(output_dir, device_ids):
        import jax

        jax.devices()
        if device_ids:
            ids = (ctypes.c_int64 * len(device_ids))(*device_ids)
            rc = lib.axon_start_nrt_profile(ids, len(device_ids))
        else:
            rc = lib.axon_start_nrt_profile(None, 0)
        if rc != 0:
            raise RuntimeError(f"axon_start_nrt_profile rc={rc}")
        try:
            yield
        finally:
            n = lib.axon_stop_nrt_profile(str(output_dir).encode())
            if n <= 0:
                print(f"ntff profile: rc={n} (no files?) dir={output_dir}", file=sys.stderr)

    return _hook


def _install_ntff_shim():
    if "antenv.axon_hooks" in sys.modules:
        return
    mod = types.ModuleType("antenv.axon_hooks")
    _hook = _make_ntff_hook()
    mod.get_axon_ntff_profile_hook = lambda: _hook
    mod.set_axon_ntff_profile_hook = lambda h: None
    sys.modules["antenv.axon_hooks"] = mod
    try:
        import antenv

        antenv.axon_hooks = mod
    except ImportError:
        pass


_install_ntff_shim()

# ---------------------------------------------------------------------------


def build_nc(s=S):
    """Build + compile the per-core Bass program (same program on all cores)."""
    nb = s // 128     # number of j-bands
    nch = s // 512    # number of 512-wide i-chunks

    nc = bacc.Bacc("TRN2", target_bir_lowering=False, debug=False, enable_asserts=False)

    xt = nc.dram_tensor("xt", [FIN, s], DT.float16, kind="ExternalInput").ap()
    w = nc.dram_tensor("w", [FIN, FOUT], DT.float16, kind="ExternalInput").ap()
    wt = nc.dram_tensor("wt", [FOUT, FIN], DT.float16, kind="ExternalInput").ap()
    a12 = nc.dram_tensor("a12", [FOUT, 2], DT.float16, kind="ExternalInput").ap()
    adjt = nc.dram_tensor("adjt", [s, s], DT.float16, kind="ExternalInput").ap()
    out = nc.dram_tensor("out", [FOUT, s], DT.float32, kind="ExternalOutput").ap()

    with tile.TileContext(nc) as tc, ExitStack() as ctx:
        _body(ctx, tc, nc, xt, w, wt, a12, adjt, out, s, nb, nch)

    nc.compile()
    return nc


def _body(ctx, tc, nc, xt, w, wt, a12, adjt, out, s, nb, nch):
    f32, f16 = DT.float32, DT.float16

    # ---------------- persistent intermediates (live through main loop) ----
    cpool = ctx.enter_context(tc.tile_pool(name="const", bufs=1))
    e1b_sb = cpool.tile([128, s], f16, tag="e1b")      # exp(f1[i]) bcast
    e1sb_sb = cpool.tile([128, s], f16, tag="e1sb")    # exp(0.2 f1[i]) bcast
    h_sb = cpool.tile([128, nb * FOUT], f16, tag="h")  # h, band b at cols [b*64, +64)
    et_sb = cpool.tile([128, nb], f32, tag="et")       # exp(0.8 f2), band b in col b
    e2s_sb = cpool.tile([128, nb], f32, tag="e2s")     # exp(0.2 f2)

    # ---------------- preamble (scoped pools, freed before main loop) ------
    with tc.tile_pool(name="pre_sb", bufs=1) as tpool:
        xt_sb = tpool.tile([FIN, s], f16, tag="xt")
        nc.sync.dma_start(xt_sb[:], xt[:])
        w65_sb = tpool.tile([FIN, FOUT + 1], f16, tag="w65")
        nc.sync.dma_start(w65_sb[:, 0:FOUT], w[:])
        wt_sb = tpool.tile([FOUT, FIN], f16, tag="wt")
        nc.sync.dma_start(wt_sb[:], wt[:])
        a12_sb = tpool.tile([FOUT, 2], f16, tag="a12")
        nc.sync.dma_start(a12_sb[:], a12[:])
        ones_sb = tpool.tile([1, 128], f16, tag="ones")
        nc.vector.memset(ones_sb[:], 1.0)
        wa_sb = tpool.tile([FIN, 2], f16, tag="wa")    # [wa1 | wa2]
        f1r_sb = tpool.tile([1, s], f16, tag="f1r")    # f1 row
        f2_sb = tpool.tile([128, nb], f32, tag="f2")   # f2, band b in col b

        # wa = W @ [a1 | a2]  (contract over FOUT)
        with tc.tile_pool(name="pre_wa", bufs=1, space="PSUM") as pwa:
            wa_ps = pwa.tile([FIN, 2], f32, tag="wa")
            nc.tensor.matmul(wa_ps[:], lhsT=wt_sb[:], rhs=a12_sb[:], start=True, stop=True)
            nc.vector.tensor_copy(wa_sb[:], wa_ps[:])
            nc.vector.tensor_copy(w65_sb[:, FOUT : FOUT + 1], wa_ps[:, 1:2])

        # f1 row chunks -> broadcast to 128 partitions -> Exp straight from PSUM
        with tc.tile_pool(name="pre_f1", bufs=2, space="PSUM") as pf1p, \
             tc.tile_pool(name="pre_bc", bufs=2, space="PSUM") as pbcp:
            for c in range(s // 512):
                pf1 = pf1p.tile([1, 512], f32, tag="f1")
                nc.tensor.matmul(pf1[:], lhsT=wa_sb[:, 0:1], rhs=xt_sb[:, bass.ts(c, 512)], start=True, stop=True)
                nc.vector.tensor_copy(f1r_sb[:, bass.ts(c, 512)], pf1[:])
            for c in range(s // 512):
                pbc = pbcp.tile([128, 512], f32, tag="bc")
                nc.tensor.matmul(pbc[:], lhsT=ones_sb[:], rhs=f1r_sb[:, bass.ts(c, 512)], start=True, stop=True)
                nc.scalar.activation(e1b_sb[:, bass.ts(c, 512)], pbc[:], AF.Exp)
                nc.scalar.activation(e1sb_sb[:, bass.ts(c, 512)], pbc[:], AF.Exp, scale=0.2)

        # [h_band | f2_band] = xt_band^T @ [W | wa2]  (one matmul per band)
        with tc.tile_pool(name="pre_h", bufs=2, space="PSUM") as phf:
            for b in range(nb):
                ph = phf.tile([128, FOUT + 1], f32, tag="hf")
                nc.tensor.matmul(ph[:], lhsT=xt_sb[:, bass.ts(b, 128)], rhs=w65_sb[:], start=True, stop=True)
                nc.vector.tensor_copy(h_sb[:, bass.ts(b, FOUT)], ph[:, 0:FOUT])
                nc.vector.tensor_copy(f2_sb[:, b : b + 1], ph[:, FOUT : FOUT + 1])

        # exp of f2 cols (small ACT ops)
        nc.scalar.activation(et_sb[:], f2_sb[:], AF.Exp, scale=0.8)
        nc.scalar.activation(e2s_sb[:], f2_sb[:], AF.Exp, scale=0.2)

    # ---------------- main loop over j-bands ----------------
    mpool = ctx.enter_context(tc.tile_pool(name="mask", bufs=4))
    tpool2 = ctx.enter_context(tc.tile_pool(name="tmax", bufs=2))
    ppool = ctx.enter_context(tc.tile_pool(name="pmat", bufs=3))
    spool = ctx.enter_context(tc.tile_pool(name="svec", bufs=8))
    hppool = ctx.enter_context(tc.tile_pool(name="hp", bufs=3))

    mainpsum = ctx.enter_context(tc.tile_pool(name="out_psum", bufs=1, space="PSUM"))
    psum_out = mainpsum.tile([FOUT, s], f32, tag="out")

    for b in range(nb):
        m_t = mpool.tile([128, s], f16, tag="m")
        nc.sync.dma_start(m_t[:], adjt[bass.ts(b, 128), :])

        # t = max(E1b * et[j], E1sb)
        t_t = tpool2.tile([128, s], f16, tag="t")
        nc.vector.scalar_tensor_tensor(
            out=t_t[:], in0=e1b_sb[:], scalar=et_sb[:, b : b + 1], in1=e1sb_sb[:],
            op0=ALU.mult, op1=ALU.max,
        )
        # P = (t * e2s[j]) * M, s[j] = sum_i P
        p_t = ppool.tile([128, s], f16, tag="p")
        s_t = spool.tile([128, 1], f32, tag="s")
        nc.vector.scalar_tensor_tensor(
            out=p_t[:], in0=t_t[:], scalar=e2s_sb[:, b : b + 1], in1=m_t[:],
            op0=ALU.mult, op1=ALU.mult, accum_out=s_t[:],
        )

        # hp = h[band] / s  -> fp16 (ACT copy with per-partition scale)
        rs_t = spool.tile([128, 1], f32, tag="rs")
        nc.vector.reciprocal(rs_t[:], s_t[:])
        hp_t = hppool.tile([128, FOUT], f16, tag="hp")
        nc.scalar.activation(hp_t[:], h_sb[:, bass.ts(b, FOUT)], AF.Copy, scale=rs_t[:])

        # out^T[o, i] += sum_j hp[j, o] * P[j, i]
        for c in range(nch):
            nc.tensor.matmul(
                psum_out[:, bass.ts(c, 512)], lhsT=hp_t[:], rhs=p_t[:, bass.ts(c, 512)],
                start=(b == 0), stop=(b == nb - 1),
            )

    # ---------------- ELU + writeout (chunked to bound SBUF) ----------------
    fpool = ctx.enter_context(tc.tile_pool(name="fin", bufs=2))
    ew = min(2048, s)
    for c in range(s // ew):
        sl = bass.ts(c, ew)
        r_t = fpool.tile([FOUT, ew], f32, tag="relu")
        nc.scalar.activation(r_t[:], psum_out[:, sl], AF.Relu)
        mn_t = fpool.tile([FOUT, ew], f32, tag="min")
        nc.vector.tensor_scalar_min(out=mn_t[:], in0=psum_out[:, sl], scalar1=0.0)
        e_t = fpool.tile([FOUT, ew], f32, tag="exp")
        nc.scalar.activation(e_t[:], mn_t[:], AF.Exp)
        f_t = fpool.tile([FOUT, ew], f32, tag="fin")
        # f = (e - 1) + r
        nc.vector.scalar_tensor_tensor(out=f_t[:], in0=e_t[:], scalar=-1.0, in1=r_t[:], op0=ALU.add, op1=ALU.add)
        nc.sync.dma_start(out[:, sl], f_t[:])


_NC_CACHE = {}


def _get_nc(s=S):
    key = (s,)
    if key not in _NC_CACHE:
        _NC_CACHE[key] = build_nc(s)
    return _NC_CACHE[key]


def kernel(input_seq, adj, W, a_1, a_2):
    """Full-input entry point: shards by head across 8 cores, returns [S, H*FOUT]."""
    global LAST_RESULTS
    X = np.asarray(input_seq)[0]          # [S, FIN] f32
    adjm = np.asarray(adj)[0]             # [S, S] int32
    Wn = np.asarray(W)                    # [H, FIN, FOUT]
    a1n = np.asarray(a_1)                 # [H, FOUT, 1]
    a2n = np.asarray(a_2)                 # [H, FOUT, 1]

    s = X.shape[0]
    xt = np.ascontiguousarray(X.T, dtype=np.float16)
    adjt = np.ascontiguousarray((adjm.T != 0)).astype(np.float16)

    nc = _get_nc(s)
    in_maps = [
        {
            "xt": xt,
            "w": np.ascontiguousarray(Wn[h], dtype=np.float16),
            "wt": np.ascontiguousarray(Wn[h].T, dtype=np.float16),
            "a12": np.ascontiguousarray(
                np.concatenate([a1n[h], a2n[h]], axis=1), dtype=np.float16
            ),
            "adjt": adjt,
        }
        for h in range(H)
    ]
    res = bass_utils.run_bass_kernel_spmd(nc, in_maps, core_ids=list(range(H)))
    LAST_RESULTS = res

    outf = np.empty((s, H * FOUT), dtype=np.float32)
    for h in range(H):
        outf[:, h * FOUT : (h + 1) * FOUT] = res.results[h]["out"].T
    return outf


# revision 15
# speedup vs baseline: 1.4018x; 1.3469x over previous
"""GAT multi-head attention layer (nn_GATMutiHeadAttLayer) on 8 Trainium2 cores.

Head-sharded: core h computes head h entirely (no collectives).

Math (per head):
  h = X @ W                       [S, FOUT]
  f1 = h @ a1, f2 = h @ a2        [S]
  e[i,j] = lrelu(f1[i] + f2[j], 0.2), masked by adj[i,j]
  attn = softmax(e, axis=i)  (denominator s[j] = sum_i)
  out = attn @ h, concat heads, ELU.

Device formulation (transposed, j on partitions; fp16 tiles):
  exp(lrelu(z)) = max(exp(z), exp(0.2 z)),  z = f1[i] + f2[j]
  u'[j,i] = max(R1b[i] * eu[j], ev[j])          (TS: mult, max — 2x/4x fast path)
     R1b = exp(0.8 f1) bcast, eu = exp(f2), ev = exp(0.2 f2)
  pb[j,i] = u' * E1sb[i]                        (TT: mult — 2x fast path)
     E1sb = exp(0.2 f1) bcast;  pb = exp(lrelu(z)) unmasked, >= 0
  pb += mask'[j,i]  (mask' in {0, -BIG}) via SWDGE accumulate-DMA (plan D)
                    or a DVE TT add against a DMA'd mask tile (plan Z fallback)
  p = relu(pb), s[j] = sum_i p   (one ACT pass: zeroes masked entries AND
                                  row-sums via the fused accumulator)
  hp = h[band] / s  (GPSIMD normalize_recip: fused divide + reciprocal)
  out^T[o,i] = sum_j hp[j,o] * p[j,i]   (PE, PSUM-accumulated over bands)
  final: ELU(out^T) -> DRAM; host transposes/concats heads.

Preamble (PE fp16):
  wa = W @ [a1|a2] via W^T;  f1 row = wa1^T @ X^T;  bcast via ones-matmul;
  R1b/E1sb exp'd straight out of PSUM.
  [h_band | f2_band] = xt_band^T @ [W | wa2]  (one N=65 matmul per band)
  eu = exp(f2), ev = exp(0.2 f2).

Host prep: X^T, W, W^T, [a1|a2] cast fp16; adj^T cast to fp16 (0/1 exact).
All model compute (matmuls, exp, masking, softmax, ELU) runs on device.
"""

import contextlib
import ctypes
import os
import sys
import types
from contextlib import ExitStack

import numpy as np

import concourse.bass as bass
import concourse.tile as tile
from concourse import bacc, mybir
from concourse import bass_utils

AF = mybir.ActivationFunctionType
ALU = mybir.AluOpType
DT = mybir.dt

S = 4096
FIN = 128
FOUT = 64
H = 8
ALPHA = 0.2

LAST_RESULTS = None  # BassKernelResults of the most recent run (for test harness)

# ---------------------------------------------------------------------------
# NTFF profile hook shim: antenv.axon_hooks is absent in this container; the
# trace=True path of run_bass_kernel_spmd imports it. Recreate it via ctypes
# against libaxon_pjrt.so (same as trn_agent_boot does).
_SO_PATH = "/opt/axon/libaxon_pjrt.so"


def _make_ntff_hook():
    try:
        lib = ctypes.CDLL(_SO_PATH)
    except OSError:
        return None
    if not hasattr(lib, "axon_start_nrt_profile"):
        return None
    lib.axon_start_nrt_profile.argtypes = [ctypes.POINTER(ctypes.c_int64), ctypes.c_size_t]
    lib.axon_start_nrt_profile.restype = ctypes.c_int64
    lib.axon_stop_nrt_profile.argtypes = [ctypes.c_char_p]
    lib.axon_stop_nrt_profile.restype = ctypes.c_int64

    @contextlib.contextmanager
    def _hook(output_dir, device_ids):
        import jax

        jax.devices()
        if device_ids:
            ids = (ctypes.c_int64 * len(device_ids))(*device_ids)
            rc = lib.axon_start_nrt_profile(ids, len(device_ids))
        else:
            rc = lib.axon_start_nrt_profile(None, 0)
        if rc != 0:
            raise RuntimeError(f"axon_start_nrt_profile rc={rc}")
        try:
            yield
        finally:
            n = lib.axon_stop_nrt_profile(str(output_dir).encode())
            if n <= 0:
                print(f"ntff profile: rc={n} (no files?) dir={output_dir}", file=sys.stderr)

    return _hook


def _install_ntff_shim():
    if "antenv.axon_hooks" in sys.modules:
        return
    mod = types.ModuleType("antenv.axon_hooks")
    _hook = _make_ntff_hook()
    mod.get_axon_ntff_profile_hook = lambda: _hook
    mod.set_axon_ntff_profile_hook = lambda h: None
    sys.modules["antenv.axon_hooks"] = mod
    try:
        import antenv

        antenv.axon_hooks = mod
    except ImportError:
        pass


_install_ntff_shim()

# ---------------------------------------------------------------------------

PLAN = os.environ.get("KERNEL_PLAN", "Z")
USE_GP = os.environ.get("KERNEL_GP", "0") == "1"


def build_nc(s=S, plan=None):
    """Build + compile the per-core Bass program (same program on all cores)."""
    plan = plan or PLAN
    nb = s // 128     # number of j-bands
    nch = s // 512    # number of 512-wide i-chunks

    nc = bacc.Bacc("TRN2", target_bir_lowering=False, debug=False, enable_asserts=False)

    xt = nc.dram_tensor("xt", [FIN, s], DT.float16, kind="ExternalInput").ap()
    w = nc.dram_tensor("w", [FIN, FOUT], DT.float16, kind="ExternalInput").ap()
    wt = nc.dram_tensor("wt", [FOUT, FIN], DT.float16, kind="ExternalInput").ap()
    a12 = nc.dram_tensor("a12", [FOUT, 2], DT.float16, kind="ExternalInput").ap()
    adjt = nc.dram_tensor("adjt", [s, s], DT.float16, kind="ExternalInput").ap()
    out = nc.dram_tensor("out", [FOUT, s], DT.float32, kind="ExternalOutput").ap()

    with tile.TileContext(nc) as tc, ExitStack() as ctx:
        _body(ctx, tc, nc, xt, w, wt, a12, adjt, out, s, nb, nch, plan)

    nc.compile()
    return nc


def _body(ctx, tc, nc, xt, w, wt, a12, adjt, out, s, nb, nch, plan):
    f32, f16 = DT.float32, DT.float16

    # ---------------- persistent intermediates (live through main loop) ----
    cpool = ctx.enter_context(tc.tile_pool(name="const", bufs=1))
    r1b_sb = cpool.tile([128, s], f16, tag="r1b")      # exp(0.8 f1[i]) bcast
    e1sb_sb = cpool.tile([128, s], f16, tag="e1sb")    # exp(0.2 f1[i]) bcast
    h_sb = cpool.tile([128, nb * FOUT], f32, tag="h")  # h (f32 for normalize_recip)
    eu_sb = cpool.tile([128, nb], f32, tag="eu")       # exp(f2), band b in col b
    ev_sb = cpool.tile([128, nb], f32, tag="ev")       # exp(0.2 f2)

    # ---------------- preamble (scoped pools, freed before main loop) ------
    with tc.tile_pool(name="pre_sb", bufs=1) as tpool:
        xt_sb = tpool.tile([FIN, s], f16, tag="xt")
        nc.sync.dma_start(xt_sb[:], xt[:])
        w65_sb = tpool.tile([FIN, FOUT + 1], f16, tag="w65")
        nc.sync.dma_start(w65_sb[:, 0:FOUT], w[:])
        wt_sb = tpool.tile([FOUT, FIN], f16, tag="wt")
        nc.sync.dma_start(wt_sb[:], wt[:])
        a12_sb = tpool.tile([FOUT, 2], f16, tag="a12")
        nc.sync.dma_start(a12_sb[:], a12[:])
        ones_sb = tpool.tile([1, 128], f16, tag="ones")
        nc.vector.memset(ones_sb[:], 1.0)
        wa_sb = tpool.tile([FIN, 2], f16, tag="wa")    # [wa1 | wa2]
        f1r_sb = tpool.tile([1, s], f16, tag="f1r")    # f1 row
        f2_sb = tpool.tile([128, nb], f32, tag="f2")   # f2, band b in col b

        # wa = W @ [a1 | a2]  (contract over FOUT)
        with tc.tile_pool(name="pre_wa", bufs=1, space="PSUM") as pwa:
            wa_ps = pwa.tile([FIN, 2], f32, tag="wa")
            nc.tensor.matmul(wa_ps[:], lhsT=wt_sb[:], rhs=a12_sb[:], start=True, stop=True)
            nc.vector.tensor_copy(wa_sb[:], wa_ps[:])
            nc.vector.tensor_copy(w65_sb[:, FOUT : FOUT + 1], wa_ps[:, 1:2])

        # [h_band | f2_band] = xt_band^T @ [W | wa2]  (one matmul per band)
        with tc.tile_pool(name="pre_h", bufs=2, space="PSUM") as phf:
            for b in range(nb):
                ph = phf.tile([128, FOUT + 1], f32, tag="hf")
                nc.tensor.matmul(ph[:], lhsT=xt_sb[:, bass.ts(b, 128)], rhs=w65_sb[:], start=True, stop=True)
                nc.vector.tensor_copy(h_sb[:, bass.ts(b, FOUT)], ph[:, 0:FOUT])
                nc.vector.tensor_copy(f2_sb[:, b : b + 1], ph[:, FOUT : FOUT + 1])

        # exp of f2 cols (small ACT ops) — early so they don't wait on bcast exps
        nc.scalar.activation(eu_sb[:], f2_sb[:], AF.Exp)
        nc.scalar.activation(ev_sb[:], f2_sb[:], AF.Exp, scale=0.2)

        # f1 row chunks -> broadcast to 128 partitions -> Exp straight from PSUM
        with tc.tile_pool(name="pre_f1", bufs=2, space="PSUM") as pf1p, \
             tc.tile_pool(name="pre_bc", bufs=2, space="PSUM") as pbcp:
            for c in range(s // 512):
                pf1 = pf1p.tile([1, 512], f32, tag="f1")
                nc.tensor.matmul(pf1[:], lhsT=wa_sb[:, 0:1], rhs=xt_sb[:, bass.ts(c, 512)], start=True, stop=True)
                nc.vector.tensor_copy(f1r_sb[:, bass.ts(c, 512)], pf1[:])
            for c in range(s // 512):
                pbc = pbcp.tile([128, 512], f32, tag="bc")
                nc.tensor.matmul(pbc[:], lhsT=ones_sb[:], rhs=f1r_sb[:, bass.ts(c, 512)], start=True, stop=True)
                nc.scalar.activation(r1b_sb[:, bass.ts(c, 512)], pbc[:], AF.Exp, scale=0.8)
                nc.scalar.activation(e1sb_sb[:, bass.ts(c, 512)], pbc[:], AF.Exp, scale=0.2)

    # ---------------- main loop over j-bands ----------------
    upool = ctx.enter_context(tc.tile_pool(name="umax", bufs=2))
    ppool = ctx.enter_context(tc.tile_pool(name="pmat", bufs=3))
    scrpool = ctx.enter_context(tc.tile_pool(name="scr", bufs=2))
    spool = ctx.enter_context(tc.tile_pool(name="svec", bufs=8))
    hppool = ctx.enter_context(tc.tile_pool(name="hp", bufs=3))
    mpool = ctx.enter_context(tc.tile_pool(name="mask", bufs=3)) if plan == "Z" else None
    qpool = ctx.enter_context(tc.tile_pool(name="qtmp", bufs=2)) if plan == "Z" else None

    mainpsum = ctx.enter_context(tc.tile_pool(name="out_psum", bufs=1, space="PSUM"))
    psum_out = mainpsum.tile([FOUT, s], f32, tag="out")

    for b in range(nb):
        # u' = max(R1b * eu[j], ev[j])   (TS, fast path)
        u_t = upool.tile([128, s], f16, tag="u")
        nc.vector.tensor_scalar(
            out=u_t[:], in0=r1b_sb[:], scalar1=eu_sb[:, b : b + 1],
            scalar2=ev_sb[:, b : b + 1], op0=ALU.mult, op1=ALU.max,
        )
        pb_t = ppool.tile([128, s], f16, tag="pb")
        if plan == "D":
            # pb = u' * E1sb (TT, unmasked exp(lrelu(z)) >= 0), then the mask
            # rides the DMA: pb += mask' with mask' in {0, -BIG}.  Masked-out
            # entries go far negative; the ACT Relu pass below zeroes them.
            nc.vector.tensor_tensor(out=pb_t[:], in0=u_t[:], in1=e1sb_sb[:], op=ALU.mult)
            nc.gpsimd.dma_start(pb_t[:], adjt[bass.ts(b, 128), :], accum_op=ALU.add)
        else:
            # plan Z: mask tile via HWDGE, two TTs on DVE (mult combine, add mask)
            m_t = mpool.tile([128, s], f16, tag="m")
            nc.sync.dma_start(m_t[:], adjt[bass.ts(b, 128), :])
            q_t = qpool.tile([128, s], f16, tag="q")
            nc.vector.tensor_tensor(out=q_t[:], in0=u_t[:], in1=e1sb_sb[:], op=ALU.mult)
            nc.vector.tensor_tensor(out=pb_t[:], in0=q_t[:], in1=m_t[:], op=ALU.add)

        # p = relu(pb) — zeroes masked entries; s[j] = sum_i p (fused accum).
        # The relu'd output is the real P consumed by the matmul.
        s_t = spool.tile([128, 1], f32, tag="s")
        p_t = scrpool.tile([128, s], f16, tag="p")
        nc.scalar.activation(p_t[:], pb_t[:], AF.Relu, accum_out=s_t[:])

        # hp = h[band] / s
        hp_t = hppool.tile([128, FOUT], f16, tag="hp")
        if USE_GP:
            # gpsimd fused divide; also writes 1/s into s_t
            nc.gpsimd.normalize_recip(hp_t[:], h_sb[:, bass.ts(b, FOUT)], s_t[:])
        else:
            rs_t = spool.tile([128, 1], f32, tag="rs")
            nc.vector.reciprocal(rs_t[:], s_t[:])
            nc.scalar.activation(hp_t[:], h_sb[:, bass.ts(b, FOUT)], AF.Copy, scale=rs_t[:])

        # out^T[o, i] += sum_j hp[j, o] * p[j, i]
        for c in range(nch):
            nc.tensor.matmul(
                psum_out[:, bass.ts(c, 512)], lhsT=hp_t[:], rhs=p_t[:, bass.ts(c, 512)],
                start=(b == 0), stop=(b == nb - 1),
            )

    # ---------------- ELU + writeout (chunked to bound SBUF) ----------------
    fpool = ctx.enter_context(tc.tile_pool(name="fin", bufs=2))
    ew = min(1024, s)
    for c in range(s // ew):
        sl = bass.ts(c, ew)
        r_t = fpool.tile([FOUT, ew], f32, tag="relu")
        nc.scalar.activation(r_t[:], psum_out[:, sl], AF.Relu)
        mn_t = fpool.tile([FOUT, ew], f32, tag="min")
        nc.vector.tensor_scalar_min(out=mn_t[:], in0=psum_out[:, sl], scalar1=0.0)
        e_t = fpool.tile([FOUT, ew], f32, tag="exp")
        nc.scalar.activation(e_t[:], mn_t[:], AF.Exp)
        f_t = fpool.tile([FOUT, ew], f32, tag="fin")
        # f = (e - 1) + r
        nc.vector.scalar_tensor_tensor(out=f_t[:], in0=e_t[:], scalar=-1.0, in1=r_t[:], op0=ALU.add, op1=ALU.add)
        nc.sync.dma_start(out[:, sl], f_t[:])


_NC_CACHE = {}


def _get_nc(s=S, plan=None):
    key = (s, plan or PLAN)
    if key not in _NC_CACHE:
        _NC_CACHE[key] = build_nc(s, plan)
    return _NC_CACHE[key]


def kernel(input_seq, adj, W, a_1, a_2):
    """Full-input entry point: shards by head across 8 cores, returns [S, H*FOUT]."""
    global LAST_RESULTS
    X = np.asarray(input_seq)[0]          # [S, FIN] f32
    adjm = np.asarray(adj)[0]             # [S, S] int32
    Wn = np.asarray(W)                    # [H, FIN, FOUT]
    a1n = np.asarray(a_1)                 # [H, FOUT, 1]
    a2n = np.asarray(a_2)                 # [H, FOUT, 1]

    s = X.shape[0]
    xt = np.ascontiguousarray(X.T, dtype=np.float16)
    # mask encoded as {0, -BIG}: p = relu(p + mask') zeroes masked-out entries
    adjt = np.where(np.ascontiguousarray(adjm.T) != 0, np.float16(0.0), np.float16(-60000.0))

    nc = _get_nc(s)
    in_maps = [
        {
            "xt": xt,
            "w": np.ascontiguousarray(Wn[h], dtype=np.float16),
            "wt": np.ascontiguousarray(Wn[h].T, dtype=np.float16),
            "a12": np.ascontiguousarray(
                np.concatenate([a1n[h], a2n[h]], axis=1), dtype=np.float16
            ),
            "adjt": adjt,
        }
        for h in range(H)
    ]
    res = bass_utils.run_bass_kernel_spmd(nc, in_maps, core_ids=list(range(H)))
    LAST_RESULTS = res

    outf = np.empty((s, H * FOUT), dtype=np.float32)
    for h in range(H):
        outf[:, h * FOUT : (h + 1) * FOUT] = res.results[h]["out"].T
    return outf
